# revision 10
# baseline (speedup 1.0000x reference)
"""TBCNN tree-convolution layer on 8 trn2 NeuronCores — Bass kernel.

Math (validated against reference):
  out[b,n] = leaky_relu(X[b,n]@w_t + P[b,n]@w_l + Q[b,n]@(w_r-w_l) + conv, 0.01)
  P[b,n] = sum_j w0[b,n,j] * nodes[b, c[b,n,j]]   (w0 = has_child)
  Q[b,n] = sum_j w1[b,n,j] * nodes[b, c[b,n,j]]   (w1 = eta_r coefficient)

The wire (axon tunnel, ~60-85MB/s, ~70-90ms per round trip) dominates, so all
per-core inputs are packed into ONE uint8 blob (int8 row-scaled nodes, int16
children, bf16 coefficients with the child scales folded in, bf16 weights) and
the output returns as uint8 with per-row f32 scales. The gather runs on-device
via indirect DMA. A tiny probe output reveals the DVE float->uint8 rounding
mode so host dequantization is exact for either mode.

Sharding: batch (tree) axis, 4 trees/core across 8 cores. Weights replicated.
"""

import numpy as np

B, N, C, D, O = 32, 512, 16, 256, 256
NCORES = 8
TPC = B // NCORES          # trees per core
ROWS = TPC * N             # per-core node rows (2048)
P = 128                    # SBUF partitions

# blob layout (bytes, per core)
OFF_NODES = 0                         # (ROWS, D) int8
OFF_SIN = OFF_NODES + ROWS * D        # (ROWS, 1) f32 node row scales
OFF_CH = OFF_SIN + ROWS * 4           # (ROWS, C) int32
OFF_W01 = OFF_CH + ROWS * C * 4       # (ROWS, 2C) bf16 (child scale folded in)
OFF_WTS = OFF_W01 + ROWS * 2 * C * 2  # (D, 3O) bf16 [w_t | w_l | w_r-w_l]
OFF_CONV = OFF_WTS + D * 3 * O * 2    # (1, O) bf16
BLOB = OFF_CONV + O * 2

_STATE = None


# ---------------------------------------------------------------- host utils

def _f32_to_bf16_bits(a: np.ndarray) -> np.ndarray:
    """f32 -> bf16 (round to nearest even), returned as uint16 bits."""
    u = np.ascontiguousarray(a, np.float32).view(np.uint32)
    rounded = u + 0x7FFF + ((u >> 16) & 1)
    return (rounded >> 16).astype(np.uint16)


def _coefs(children: np.ndarray):
    """w0 (has-child mask) and w1 (eta_r) per (b, n, j)."""
    has = children > 0
    ns = has.sum(-1)
    a = np.where(ns > 1, 1.0 / np.maximum(ns - 1, 1), 0.0)
    bco = np.where(ns == 1, 0.5, 0.0)
    jar = np.arange(C, dtype=np.float32)
    w0 = has.astype(np.float32)
    w1 = (has * (a[..., None] * jar + bco[..., None] * (jar == 0))).astype(np.float32)
    return w0, w1


# ---------------------------------------------------------------- bass kernel

def _build_bass():
    from contextlib import ExitStack
    import concourse.bacc as bacc
    import concourse.bass as bass
    import concourse.tile as tile
    from concourse import mybir
    from concourse.masks import make_identity

    dt = mybir.dt
    nc = bacc.Bacc(
        "TRN2",
        target_bir_lowering=False,
        debug=False,
        num_devices=NCORES,
    )

    blob_d = nc.dram_tensor("blob", [BLOB], dt.uint8, kind="ExternalInput")
    out_d = nc.dram_tensor("outq", [ROWS, O], dt.uint8, kind="ExternalOutput")
    rsc_d = nc.dram_tensor("rscale", [ROWS, 1], dt.float32, kind="ExternalOutput")
    probe_d = nc.dram_tensor("probe", [1, 8], dt.uint8, kind="ExternalOutput")

    bap = blob_d.ap()
    nodes_v = bap[OFF_NODES:OFF_SIN].bitcast(dt.int8).rearrange("(n d) -> n d", d=D)
    sin_v = bap[OFF_SIN:OFF_CH].bitcast(dt.float32).rearrange("(n d) -> n d", d=1)
    ch_v = bap[OFF_CH:OFF_W01].bitcast(dt.int32).rearrange("(n c) -> n c", c=C)
    w01_v = bap[OFF_W01:OFF_WTS].bitcast(dt.bfloat16).rearrange(
        "(n c) -> n c", c=2 * C)
    wts_v = bap[OFF_WTS:OFF_CONV].bitcast(dt.bfloat16).rearrange(
        "(d o) -> d o", o=3 * O)
    conv_v = bap[OFF_CONV:BLOB].bitcast(dt.bfloat16).rearrange("(a o) -> a o", o=O)

    NCHUNK = ROWS // P           # 16 chunks of 128 nodes
    CPT = N // P                 # chunks per tree (4)

    with tile.TileContext(nc) as tc, ExitStack() as ctx:
        wpool = ctx.enter_context(tc.tile_pool(name="w", bufs=1))
        wts_sb = wpool.tile([P, 2 * 3 * O], dt.bfloat16)
        # [:, :768] = weight rows 0..127, [:, 768:] = rows 128..255 of [w_t|w_l|w_rl]
        nc.sync.dma_start(wts_sb[:, 0:768], wts_v[0:P, :])
        nc.sync.dma_start(wts_sb[:, 768:1536], wts_v[P : 2 * P, :])
        conv_sb = wpool.tile([1, O], dt.bfloat16)
        nc.sync.dma_start(conv_sb[:], conv_v[:])
        ones_sb = wpool.tile([1, P], dt.bfloat16)
        nc.vector.memset(ones_sb[:], 1.0)
        ident_bf = wpool.tile([P, P], dt.bfloat16)
        make_identity(nc, ident_bf[:])
        ident_f32 = wpool.tile([P, P], dt.float32)
        make_identity(nc, ident_f32[:])

        # rounding-mode probe: convert known constants f32 -> u8 on the DVE
        PROBE_VALS = [130.5, 131.5, 126.5, 130.25, 130.75, 1.5, 254.5, 128.0]
        probe_f = wpool.tile([1, 8], dt.float32)
        for i, v in enumerate(PROBE_VALS):
            nc.vector.memset(probe_f[:, i : i + 1], v)
        probe_u8 = wpool.tile([1, 8], dt.uint8)
        nc.vector.tensor_scalar(
            out=probe_u8[:], in0=probe_f[:], scalar1=1.0, scalar2=None,
            op0=mybir.AluOpType.mult,
        )
        nc.sync.dma_start(probe_d[:], probe_u8[:])

        lpool = ctx.enter_context(tc.tile_pool(name="loads", bufs=3))
        epool = ctx.enter_context(tc.tile_pool(name="emb", bufs=4))
        apool = ctx.enter_context(tc.tile_pool(name="acc", bufs=2))
        tpool = ctx.enter_context(tc.tile_pool(name="trans", bufs=2))
        opool = ctx.enter_context(tc.tile_pool(name="outs", bufs=3))
        pspool = ctx.enter_context(tc.tile_pool(name="psum", bufs=2, space="PSUM"))
        pstp = ctx.enter_context(tc.tile_pool(name="psumt", bufs=1, space="PSUM"))

        for chunk in range(NCHUNK):
            r0 = chunk * P
            tree = chunk // CPT

            x_i8 = lpool.tile([P, D], dt.int8, tag="x")
            nc.sync.dma_start(x_i8[:], nodes_v[r0 : r0 + P, :])
            sin_sb = lpool.tile([P, 1], dt.float32, tag="sin")
            nc.sync.dma_start(sin_sb[:], sin_v[r0 : r0 + P, :])
            ch_sb = lpool.tile([P, C], dt.int32, tag="ch")
            nc.sync.dma_start(ch_sb[:], ch_v[r0 : r0 + P, :])
            w01_bf = lpool.tile([P, 2 * C], dt.bfloat16, tag="w01bf")
            nc.sync.dma_start(w01_bf[:], w01_v[r0 : r0 + P, :])
            w01_sb = lpool.tile([P, 2 * C], dt.float32, tag="w01")
            nc.vector.tensor_copy(w01_sb[:], w01_bf[:])

            pacc = apool.tile([P, D], dt.float32, tag="pacc")
            qacc = apool.tile([P, D], dt.float32, tag="qacc")
            for j in range(C):
                emb = epool.tile([P, D], dt.int8, tag="emb")
                nc.gpsimd.indirect_dma_start(
                    out=emb[:],
                    out_offset=None,
                    in_=nodes_v,
                    in_offset=bass.IndirectOffsetOnAxis(ap=ch_sb[:, j : j + 1], axis=0),
                    element_offset=tree * N * D,
                )
                if j == 0:
                    nc.vector.tensor_scalar(
                        out=pacc[:], in0=emb[:], scalar1=w01_sb[:, 0:1],
                        scalar2=None, op0=mybir.AluOpType.mult,
                    )
                    nc.vector.tensor_scalar(
                        out=qacc[:], in0=emb[:], scalar1=w01_sb[:, C : C + 1],
                        scalar2=None, op0=mybir.AluOpType.mult,
                    )
                else:
                    tmp_p = epool.tile([P, D], dt.float32, tag="tmp_p")
                    nc.scalar.activation(
                        out=tmp_p[:], in_=emb[:],
                        func=mybir.ActivationFunctionType.Copy,
                        scale=w01_sb[:, j : j + 1],
                    )
                    nc.vector.tensor_add(pacc[:], pacc[:], tmp_p[:])
                    tmp_q = epool.tile([P, D], dt.float32, tag="tmp_q")
                    nc.vector.tensor_scalar(
                        out=tmp_q[:], in0=emb[:], scalar1=w01_sb[:, C + j : C + j + 1],
                        scalar2=None, op0=mybir.AluOpType.mult,
                    )
                    nc.vector.tensor_add(qacc[:], qacc[:], tmp_q[:])

            # dequantize parent rows: xf = x_i8 * row_scale
            xf = lpool.tile([P, D], dt.bfloat16, tag="xf")
            nc.vector.tensor_scalar(
                out=xf[:], in0=x_i8[:], scalar1=sin_sb[:, 0:1],
                scalar2=None, op0=mybir.AluOpType.mult,
            )

            # transpose X, P, Q into (d, n) layout for the output matmuls
            xt = tpool.tile([P, D], dt.bfloat16, tag="xt")
            pt = tpool.tile([P, D], dt.bfloat16, tag="pt")
            qt = tpool.tile([P, D], dt.bfloat16, tag="qt")
            for dc in range(2):
                sl = slice(dc * P, (dc + 1) * P)
                tp_x = pstp.tile([P, P], dt.bfloat16, tag="tp_x")
                nc.tensor.transpose(out=tp_x[:], in_=xf[:, sl], identity=ident_bf[:])
                nc.scalar.copy(xt[:, sl], tp_x[:])
                tp_p = pstp.tile([P, P], dt.float32, tag="tp_p")
                nc.tensor.transpose(out=tp_p[:], in_=pacc[:, sl], identity=ident_f32[:])
                nc.scalar.copy(pt[:, sl], tp_p[:])
                tp_q = pstp.tile([P, P], dt.float32, tag="tp_q")
                nc.tensor.transpose(out=tp_q[:], in_=qacc[:, sl], identity=ident_f32[:])
                nc.scalar.copy(qt[:, sl], tp_q[:])

            # out[n, o] = Xt.T@w_t + Pt.T@w_l + Qt.T@w_rl + ones.T@conv
            out_ps = pspool.tile([P, O], dt.float32, tag="ops")
            nc.tensor.matmul(out=out_ps[:], lhsT=xt[:, 0:P], rhs=wts_sb[:, 0:256],
                             start=True, stop=False)
            nc.tensor.matmul(out=out_ps[:], lhsT=xt[:, P:D], rhs=wts_sb[:, 768:1024],
                             start=False, stop=False)
            nc.tensor.matmul(out=out_ps[:], lhsT=pt[:, 0:P], rhs=wts_sb[:, 256:512],
                             start=False, stop=False)
            nc.tensor.matmul(out=out_ps[:], lhsT=pt[:, P:D], rhs=wts_sb[:, 1024:1280],
                             start=False, stop=False)
            nc.tensor.matmul(out=out_ps[:], lhsT=qt[:, 0:P], rhs=wts_sb[:, 512:768],
                             start=False, stop=False)
            nc.tensor.matmul(out=out_ps[:], lhsT=qt[:, P:D], rhs=wts_sb[:, 1280:1536],
                             start=False, stop=False)
            nc.tensor.matmul(out=out_ps[:], lhsT=ones_sb[:], rhs=conv_sb[:],
                             start=False, stop=True)

            # leaky relu, then per-row uint8 quantization
            oful = opool.tile([P, O], dt.float32, tag="oful")
            small = opool.tile([P, O], dt.float32, tag="small")
            nc.scalar.mul(small[:], out_ps[:], 0.01)
            nc.vector.tensor_tensor(
                out=oful[:], in0=out_ps[:], in1=small[:], op=mybir.AluOpType.max,
            )
            rmax = opool.tile([P, 1], dt.float32, tag="rmax")
            nc.vector.reduce_max(
                out=rmax[:], in_=oful[:], axis=mybir.AxisListType.X,
                apply_absolute_value=True,
            )
            rmax_c = opool.tile([P, 1], dt.float32, tag="rmaxc")
            nc.vector.tensor_scalar_max(rmax_c[:], rmax[:], 1e-30)
            inv = opool.tile([P, 1], dt.float32, tag="inv")
            nc.vector.reciprocal(inv[:], rmax_c[:])
            inv127 = opool.tile([P, 1], dt.float32, tag="inv127")
            nc.vector.tensor_scalar_mul(inv127[:], inv[:], 127.0)
            q_u8 = opool.tile([P, O], dt.uint8, tag="qu8")
            nc.vector.tensor_scalar(
                out=q_u8[:], in0=oful[:], scalar1=inv127[:, 0:1], scalar2=128.5,
                op0=mybir.AluOpType.mult, op1=mybir.AluOpType.add,
            )
            nc.sync.dma_start(out_d[r0 : r0 + P, :], q_u8[:])
            nc.sync.dma_start(rsc_d[r0 : r0 + P, :], rmax_c[:])

    nc.compile()
    if not nc.is_finalized():
        nc.finalize()
    return nc


# ---------------------------------------------------------------- host pack

def _pack_inputs(inputs):
    nodes = np.ascontiguousarray(np.asarray(inputs["nodes"], np.float32))
    children = np.asarray(inputs["children"]).astype(np.int64)
    w_t = np.asarray(inputs["w_t"], np.float32)
    w_l = np.asarray(inputs["w_l"], np.float32)
    w_r = np.asarray(inputs["w_r"], np.float32)
    conv = np.asarray(inputs["conv"], np.float32)

    # int8 row-scaled nodes (host-side round-to-nearest)
    absmax = np.abs(nodes).max(axis=-1)                      # (B, N)
    s_in = (absmax / 127.0 + 1e-30).astype(np.float32)
    q_nodes = np.rint(nodes * (1.0 / s_in)[..., None]).astype(np.int8)

    # coefficients with the gathered child's dequant scale folded in
    w0, w1 = _coefs(children)
    bidx = np.arange(B)[:, None, None]
    s_child = s_in[bidx, children]                           # (B, N, C)
    w01 = _f32_to_bf16_bits(
        np.concatenate([w0 * s_child, w1 * s_child], axis=-1))  # (B,N,2C) u16

    wts = _f32_to_bf16_bits(np.concatenate([w_t, w_l, w_r - w_l], axis=1))
    conv_bf = _f32_to_bf16_bits(conv)

    blob = np.empty((NCORES, BLOB), np.uint8)
    blob[:, OFF_NODES:OFF_SIN] = q_nodes.reshape(NCORES, -1).view(np.uint8)
    blob[:, OFF_SIN:OFF_CH] = s_in.reshape(NCORES, -1).view(np.uint8)
    blob[:, OFF_CH:OFF_W01] = (
        children.astype(np.int32).reshape(NCORES, -1).view(np.uint8))
    blob[:, OFF_W01:OFF_WTS] = w01.reshape(NCORES, -1).view(np.uint8)
    blob[:, OFF_WTS:OFF_CONV] = wts.reshape(1, -1).view(np.uint8)
    blob[:, OFF_CONV:BLOB] = conv_bf.reshape(1, -1).view(np.uint8)
    return blob


# ---------------------------------------------------------------- jax glue

def _build_exec():
    import jax
    from jax.sharding import Mesh, PartitionSpec
    from jax.experimental.shard_map import shard_map
    from concourse import bass2jax, mybir

    nc = _build_bass()
    bass2jax.install_neuronx_cc_hook()

    in_names, out_names, out_avals = [], [], []
    partition_name = (
        nc.partition_id_tensor.name if nc.partition_id_tensor is not None else None
    )
    for alloc in nc.m.functions[0].allocations:
        if not isinstance(alloc, mybir.MemoryLocationSet):
            continue
        name = alloc.memorylocations[0].name
        if alloc.kind == "ExternalInput":
            if name != partition_name:
                in_names.append(name)
        elif alloc.kind == "ExternalOutput":
            out_names.append(name)
            out_avals.append(
                jax.core.ShapedArray(
                    tuple(alloc.tensor_shape), mybir.dt.np(alloc.dtype)
                )
            )
    if partition_name is not None:
        in_names.append(partition_name)

    devices = jax.devices()[:NCORES]
    mesh = Mesh(np.asarray(devices), ("core",))

    def _body(*args):
        operands = list(args)
        if partition_name is not None:
            operands.append(bass2jax.partition_id_tensor())
        outs = bass2jax._bass_exec_p.bind(
            *operands,
            out_avals=tuple(out_avals),
            in_names=tuple(in_names),
            out_names=tuple(out_names),
            lowering_input_output_aliases=(),
            sim_require_finite=True,
            sim_require_nnan=True,
            nc=nc,
        )
        return tuple(outs)

    n_real = len(in_names) - (1 if partition_name else 0)
    in_specs = (PartitionSpec("core"),) * n_real
    out_specs = (PartitionSpec("core"),) * len(out_names)
    fn = jax.jit(
        shard_map(_body, mesh=mesh, in_specs=in_specs, out_specs=out_specs,
                  check_rep=False)
    )
    return {
        "fn": fn,
        "in_names": in_names[:n_real],
        "out_names": out_names,
    }


# ---------------------------------------------------------------- entry point

def kernel(**inputs):
    global _STATE

    blob = _pack_inputs(inputs)

    if _STATE is None:
        _STATE = _build_exec()

    outs = _STATE["fn"](blob.reshape(NCORES * BLOB))
    by_name = dict(zip(_STATE["out_names"], outs))
    q = np.asarray(by_name["outq"])                     # (8*ROWS, 256) u8
    rsc = np.asarray(by_name["rscale"])                 # (8*ROWS, 1) f32
    probe = np.asarray(by_name["probe"]).reshape(NCORES, 8)

    # probe[1] == 131 iff float->u8 conversion truncates (floor for positives):
    # then u8 = floor(x*127/rmax + 128.5) = rtn(x*127/rmax) + 128 -> offset 128.
    # A round-to-nearest convert gives u8 = floor(x*127/rmax) + 129 for
    # non-tie values -> offset 128.5.
    off = 128.0 if int(probe[0, 1]) == 131 else 128.5
    scale = (rsc.astype(np.float32) / 127.0)
    out = (q.astype(np.float32) - off) * scale
    return out.reshape(B, N, O).astype(np.float32)


# revision 14
# speedup vs baseline: 1.4024x; 1.4024x over previous
"""TBCNN tree-convolution layer on 8 trn2 NeuronCores — Bass kernel.

Math (validated against reference):
  out[b,n] = leaky_relu(X[b,n]@w_t + P[b,n]@w_l + Q[b,n]@(w_r-w_l) + conv, 0.01)
  P[b,n] = sum_j w0[b,n,j] * nodes[b, c[b,n,j]]   (w0 = has_child)
  Q[b,n] = sum_j w1[b,n,j] * nodes[b, c[b,n,j]]   (w1 = eta_r coefficient)

The wire (axon tunnel, ~60-85MB/s, ~70-90ms per round trip) dominates, so all
per-core inputs are packed into ONE uint8 blob (int8 row-scaled nodes, int16
children, bf16 coefficients with the child scales folded in, bf16 weights) and
the output returns as uint8 with per-row f32 scales. The gather runs on-device
via indirect DMA. A tiny probe output reveals the DVE float->uint8 rounding
mode so host dequantization is exact for either mode.

Sharding: batch (tree) axis, 4 trees/core across 8 cores. Weights replicated.
"""

import numpy as np

B, N, C, D, O = 32, 512, 16, 256, 256
NCORES = 8
TPC = B // NCORES          # trees per core
ROWS = TPC * N             # per-core node rows (2048)
P = 128                    # SBUF partitions

# blob layout (bytes, per core)
OFF_NODES = 0                         # (ROWS, D) int8
OFF_SIN = OFF_NODES + ROWS * D        # (ROWS, 1) f32 node row scales
OFF_CH = OFF_SIN + ROWS * 4           # (ROWS, C) int32
OFF_W01 = OFF_CH + ROWS * C * 4       # (ROWS, 2C) bf16 (child scale folded in)
OFF_WTS = OFF_W01 + ROWS * 2 * C * 2  # (D, 3O) bf16 [w_t | w_l | w_r-w_l]
OFF_CONV = OFF_WTS + D * 3 * O * 2    # (1, O) bf16
BLOB = OFF_CONV + O * 2

_STATE = None


# ---------------------------------------------------------------- host utils

def _f32_to_bf16_bits(a: np.ndarray) -> np.ndarray:
    """f32 -> bf16 (round to nearest even), returned as uint16 bits."""
    u = np.ascontiguousarray(a, np.float32).view(np.uint32)
    rounded = u + 0x7FFF + ((u >> 16) & 1)
    return (rounded >> 16).astype(np.uint16)


def _coefs(children: np.ndarray):
    """w0 (has-child mask) and w1 (eta_r) per (b, n, j)."""
    has = children > 0
    ns = has.sum(-1)
    a = np.where(ns > 1, 1.0 / np.maximum(ns - 1, 1), 0.0)
    bco = np.where(ns == 1, 0.5, 0.0)
    jar = np.arange(C, dtype=np.float32)
    w0 = has.astype(np.float32)
    w1 = (has * (a[..., None] * jar + bco[..., None] * (jar == 0))).astype(np.float32)
    return w0, w1


# ---------------------------------------------------------------- bass kernel

def _build_bass():
    from contextlib import ExitStack
    import concourse.bacc as bacc
    import concourse.bass as bass
    import concourse.tile as tile
    from concourse import mybir
    from concourse.masks import make_identity

    dt = mybir.dt
    nc = bacc.Bacc(
        "TRN2",
        target_bir_lowering=False,
        debug=False,
        num_devices=NCORES,
    )

    blob_d = nc.dram_tensor("blob", [BLOB], dt.uint8, kind="ExternalInput")
    # packed output: rows 0..ROWS-1 = [256B u8 quantized | 4B f32 row scale],
    # row ROWS = probe bytes
    outb_d = nc.dram_tensor("outb", [ROWS + 1, O + 4], dt.uint8,
                            kind="ExternalOutput")

    bap = blob_d.ap()
    nodes_v = bap[OFF_NODES:OFF_SIN].bitcast(dt.int8).rearrange("(n d) -> n d", d=D)
    sin_v = bap[OFF_SIN:OFF_CH].bitcast(dt.float32).rearrange("(n d) -> n d", d=1)
    ch_v = bap[OFF_CH:OFF_W01].bitcast(dt.int32).rearrange("(n c) -> n c", c=C)
    w01_v = bap[OFF_W01:OFF_WTS].bitcast(dt.bfloat16).rearrange(
        "(n c) -> n c", c=2 * C)
    wts_v = bap[OFF_WTS:OFF_CONV].bitcast(dt.bfloat16).rearrange(
        "(d o) -> d o", o=3 * O)
    conv_v = bap[OFF_CONV:BLOB].bitcast(dt.bfloat16).rearrange("(a o) -> a o", o=O)

    NCHUNK = ROWS // P           # 16 chunks of 128 nodes
    CPT = N // P                 # chunks per tree (4)

    with tile.TileContext(nc) as tc, ExitStack() as ctx:
        wpool = ctx.enter_context(tc.tile_pool(name="w", bufs=1))
        wts_sb = wpool.tile([P, 2 * 3 * O], dt.bfloat16)
        # [:, :768] = weight rows 0..127, [:, 768:] = rows 128..255 of [w_t|w_l|w_rl]
        nc.sync.dma_start(wts_sb[:, 0:768], wts_v[0:P, :])
        nc.sync.dma_start(wts_sb[:, 768:1536], wts_v[P : 2 * P, :])
        conv_sb = wpool.tile([1, O], dt.bfloat16)
        nc.sync.dma_start(conv_sb[:], conv_v[:])
        ones_sb = wpool.tile([1, P], dt.bfloat16)
        nc.vector.memset(ones_sb[:], 1.0)
        ident_bf = wpool.tile([P, P], dt.bfloat16)
        make_identity(nc, ident_bf[:])
        ident_f32 = wpool.tile([P, P], dt.float32)
        make_identity(nc, ident_f32[:])

        # rounding-mode probe: convert known constants f32 -> u8 on the DVE
        PROBE_VALS = [130.5, 131.5, 126.5, 130.25, 130.75, 1.5, 254.5, 128.0]
        probe_f = wpool.tile([1, 8], dt.float32)
        for i, v in enumerate(PROBE_VALS):
            nc.vector.memset(probe_f[:, i : i + 1], v)
        probe_u8 = wpool.tile([1, 8], dt.uint8)
        nc.vector.tensor_scalar(
            out=probe_u8[:], in0=probe_f[:], scalar1=1.0, scalar2=None,
            op0=mybir.AluOpType.mult,
        )
        nc.sync.dma_start(outb_d[ROWS : ROWS + 1, 0:8], probe_u8[:])

        lpool = ctx.enter_context(tc.tile_pool(name="loads", bufs=3))
        epool = ctx.enter_context(tc.tile_pool(name="emb", bufs=4))
        apool = ctx.enter_context(tc.tile_pool(name="acc", bufs=2))
        tpool = ctx.enter_context(tc.tile_pool(name="trans", bufs=2))
        opool = ctx.enter_context(tc.tile_pool(name="outs", bufs=3))
        pspool = ctx.enter_context(tc.tile_pool(name="psum", bufs=2, space="PSUM"))
        pstp = ctx.enter_context(tc.tile_pool(name="psumt", bufs=1, space="PSUM"))

        for chunk in range(NCHUNK):
            r0 = chunk * P
            tree = chunk // CPT

            x_i8 = lpool.tile([P, D], dt.int8, tag="x")
            nc.sync.dma_start(x_i8[:], nodes_v[r0 : r0 + P, :])
            sin_sb = lpool.tile([P, 1], dt.float32, tag="sin")
            nc.sync.dma_start(sin_sb[:], sin_v[r0 : r0 + P, :])
            ch_sb = lpool.tile([P, C], dt.int32, tag="ch")
            nc.sync.dma_start(ch_sb[:], ch_v[r0 : r0 + P, :])
            w01_bf = lpool.tile([P, 2 * C], dt.bfloat16, tag="w01bf")
            nc.sync.dma_start(w01_bf[:], w01_v[r0 : r0 + P, :])
            w01_sb = lpool.tile([P, 2 * C], dt.float32, tag="w01")
            nc.vector.tensor_copy(w01_sb[:], w01_bf[:])

            pacc = apool.tile([P, D], dt.float32, tag="pacc")
            qacc = apool.tile([P, D], dt.float32, tag="qacc")
            for j in range(C):
                emb = epool.tile([P, D], dt.int8, tag="emb")
                nc.gpsimd.indirect_dma_start(
                    out=emb[:],
                    out_offset=None,
                    in_=nodes_v,
                    in_offset=bass.IndirectOffsetOnAxis(ap=ch_sb[:, j : j + 1], axis=0),
                    element_offset=tree * N * D,
                )
                if j == 0:
                    nc.vector.tensor_scalar(
                        out=pacc[:], in0=emb[:], scalar1=w01_sb[:, 0:1],
                        scalar2=None, op0=mybir.AluOpType.mult,
                    )
                    nc.vector.tensor_scalar(
                        out=qacc[:], in0=emb[:], scalar1=w01_sb[:, C : C + 1],
                        scalar2=None, op0=mybir.AluOpType.mult,
                    )
                else:
                    tmp_p = epool.tile([P, D], dt.float32, tag="tmp_p")
                    nc.scalar.activation(
                        out=tmp_p[:], in_=emb[:],
                        func=mybir.ActivationFunctionType.Copy,
                        scale=w01_sb[:, j : j + 1],
                    )
                    nc.vector.tensor_add(pacc[:], pacc[:], tmp_p[:])
                    tmp_q = epool.tile([P, D], dt.float32, tag="tmp_q")
                    nc.vector.tensor_scalar(
                        out=tmp_q[:], in0=emb[:], scalar1=w01_sb[:, C + j : C + j + 1],
                        scalar2=None, op0=mybir.AluOpType.mult,
                    )
                    nc.vector.tensor_add(qacc[:], qacc[:], tmp_q[:])

            # dequantize parent rows: xf = x_i8 * row_scale
            xf = lpool.tile([P, D], dt.bfloat16, tag="xf")
            nc.vector.tensor_scalar(
                out=xf[:], in0=x_i8[:], scalar1=sin_sb[:, 0:1],
                scalar2=None, op0=mybir.AluOpType.mult,
            )

            # transpose X, P, Q into (d, n) layout for the output matmuls
            xt = tpool.tile([P, D], dt.bfloat16, tag="xt")
            pt = tpool.tile([P, D], dt.bfloat16, tag="pt")
            qt = tpool.tile([P, D], dt.bfloat16, tag="qt")
            for dc in range(2):
                sl = slice(dc * P, (dc + 1) * P)
                tp_x = pstp.tile([P, P], dt.bfloat16, tag="tp_x")
                nc.tensor.transpose(out=tp_x[:], in_=xf[:, sl], identity=ident_bf[:])
                nc.scalar.copy(xt[:, sl], tp_x[:])
                tp_p = pstp.tile([P, P], dt.float32, tag="tp_p")
                nc.tensor.transpose(out=tp_p[:], in_=pacc[:, sl], identity=ident_f32[:])
                nc.scalar.copy(pt[:, sl], tp_p[:])
                tp_q = pstp.tile([P, P], dt.float32, tag="tp_q")
                nc.tensor.transpose(out=tp_q[:], in_=qacc[:, sl], identity=ident_f32[:])
                nc.scalar.copy(qt[:, sl], tp_q[:])

            # out[n, o] = Xt.T@w_t + Pt.T@w_l + Qt.T@w_rl + ones.T@conv
            out_ps = pspool.tile([P, O], dt.float32, tag="ops")
            nc.tensor.matmul(out=out_ps[:], lhsT=xt[:, 0:P], rhs=wts_sb[:, 0:256],
                             start=True, stop=False)
            nc.tensor.matmul(out=out_ps[:], lhsT=xt[:, P:D], rhs=wts_sb[:, 768:1024],
                             start=False, stop=False)
            nc.tensor.matmul(out=out_ps[:], lhsT=pt[:, 0:P], rhs=wts_sb[:, 256:512],
                             start=False, stop=False)
            nc.tensor.matmul(out=out_ps[:], lhsT=pt[:, P:D], rhs=wts_sb[:, 1024:1280],
                             start=False, stop=False)
            nc.tensor.matmul(out=out_ps[:], lhsT=qt[:, 0:P], rhs=wts_sb[:, 512:768],
                             start=False, stop=False)
            nc.tensor.matmul(out=out_ps[:], lhsT=qt[:, P:D], rhs=wts_sb[:, 1280:1536],
                             start=False, stop=False)
            nc.tensor.matmul(out=out_ps[:], lhsT=ones_sb[:], rhs=conv_sb[:],
                             start=False, stop=True)

            # leaky relu, then per-row uint8 quantization
            oful = opool.tile([P, O], dt.float32, tag="oful")
            small = opool.tile([P, O], dt.float32, tag="small")
            nc.scalar.mul(small[:], out_ps[:], 0.01)
            nc.vector.tensor_tensor(
                out=oful[:], in0=out_ps[:], in1=small[:], op=mybir.AluOpType.max,
            )
            rmax = opool.tile([P, 1], dt.float32, tag="rmax")
            nc.vector.reduce_max(
                out=rmax[:], in_=oful[:], axis=mybir.AxisListType.X,
                apply_absolute_value=True,
            )
            rmax_c = opool.tile([P, 1], dt.float32, tag="rmaxc")
            nc.vector.tensor_scalar_max(rmax_c[:], rmax[:], 1e-30)
            inv = opool.tile([P, 1], dt.float32, tag="inv")
            nc.vector.reciprocal(inv[:], rmax_c[:])
            inv127 = opool.tile([P, 1], dt.float32, tag="inv127")
            nc.vector.tensor_scalar_mul(inv127[:], inv[:], 127.0)
            q_u8 = opool.tile([P, O], dt.uint8, tag="qu8")
            nc.vector.tensor_scalar(
                out=q_u8[:], in0=oful[:], scalar1=inv127[:, 0:1], scalar2=128.5,
                op0=mybir.AluOpType.mult, op1=mybir.AluOpType.add,
            )
            nc.sync.dma_start(outb_d[r0 : r0 + P, 0:O], q_u8[:])
            nc.sync.dma_start(
                outb_d[r0 : r0 + P, O : O + 4], rmax_c[:].bitcast(dt.uint8))

    nc.compile()
    if not nc.is_finalized():
        nc.finalize()
    return nc


# ---------------------------------------------------------------- host pack

def _pack_inputs(inputs):
    nodes = np.ascontiguousarray(np.asarray(inputs["nodes"], np.float32))
    children = np.asarray(inputs["children"]).astype(np.int64)
    w_t = np.asarray(inputs["w_t"], np.float32)
    w_l = np.asarray(inputs["w_l"], np.float32)
    w_r = np.asarray(inputs["w_r"], np.float32)
    conv = np.asarray(inputs["conv"], np.float32)

    # int8 row-scaled nodes (host-side round-to-nearest)
    absmax = np.abs(nodes).max(axis=-1)                      # (B, N)
    s_in = (absmax / 127.0 + 1e-30).astype(np.float32)
    q_nodes = np.rint(nodes * (1.0 / s_in)[..., None]).astype(np.int8)

    # coefficients with the gathered child's dequant scale folded in
    w0, w1 = _coefs(children)
    bidx = np.arange(B)[:, None, None]
    s_child = s_in[bidx, children]                           # (B, N, C)
    w01 = _f32_to_bf16_bits(
        np.concatenate([w0 * s_child, w1 * s_child], axis=-1))  # (B,N,2C) u16

    wts = _f32_to_bf16_bits(np.concatenate([w_t, w_l, w_r - w_l], axis=1))
    conv_bf = _f32_to_bf16_bits(conv)

    blob = np.empty((NCORES, BLOB), np.uint8)
    blob[:, OFF_NODES:OFF_SIN] = q_nodes.reshape(NCORES, -1).view(np.uint8)
    blob[:, OFF_SIN:OFF_CH] = s_in.reshape(NCORES, -1).view(np.uint8)
    blob[:, OFF_CH:OFF_W01] = (
        children.astype(np.int32).reshape(NCORES, -1).view(np.uint8))
    blob[:, OFF_W01:OFF_WTS] = w01.reshape(NCORES, -1).view(np.uint8)
    blob[:, OFF_WTS:OFF_CONV] = wts.reshape(1, -1).view(np.uint8)
    blob[:, OFF_CONV:BLOB] = conv_bf.reshape(1, -1).view(np.uint8)
    return blob


# ---------------------------------------------------------------- jax glue

def _build_exec():
    import jax
    from jax.sharding import Mesh, PartitionSpec
    from jax.experimental.shard_map import shard_map
    from concourse import bass2jax, mybir

    nc = _build_bass()
    bass2jax.install_neuronx_cc_hook()

    in_names, out_names, out_avals = [], [], []
    partition_name = (
        nc.partition_id_tensor.name if nc.partition_id_tensor is not None else None
    )
    for alloc in nc.m.functions[0].allocations:
        if not isinstance(alloc, mybir.MemoryLocationSet):
            continue
        name = alloc.memorylocations[0].name
        if alloc.kind == "ExternalInput":
            if name != partition_name:
                in_names.append(name)
        elif alloc.kind == "ExternalOutput":
            out_names.append(name)
            out_avals.append(
                jax.core.ShapedArray(
                    tuple(alloc.tensor_shape), mybir.dt.np(alloc.dtype)
                )
            )
    if partition_name is not None:
        in_names.append(partition_name)

    devices = jax.devices()[:NCORES]
    mesh = Mesh(np.asarray(devices), ("core",))

    def _body(*args):
        operands = list(args)
        if partition_name is not None:
            operands.append(bass2jax.partition_id_tensor())
        outs = bass2jax._bass_exec_p.bind(
            *operands,
            out_avals=tuple(out_avals),
            in_names=tuple(in_names),
            out_names=tuple(out_names),
            lowering_input_output_aliases=(),
            sim_require_finite=True,
            sim_require_nnan=True,
            nc=nc,
        )
        return tuple(outs)

    n_real = len(in_names) - (1 if partition_name else 0)
    in_specs = (PartitionSpec("core"),) * n_real
    out_specs = (PartitionSpec("core"),) * len(out_names)
    fn = jax.jit(
        shard_map(_body, mesh=mesh, in_specs=in_specs, out_specs=out_specs,
                  check_rep=False)
    )
    return {
        "fn": fn,
        "in_names": in_names[:n_real],
        "out_names": out_names,
    }


# ---------------------------------------------------------------- entry point

def kernel(**inputs):
    global _STATE

    blob = _pack_inputs(inputs)

    if _STATE is None:
        _STATE = _build_exec()

    outs = _STATE["fn"](blob.reshape(NCORES * BLOB))
    outb = np.asarray(outs[0]).reshape(NCORES, ROWS + 1, O + 4)
    return _unpack_output(outb)


def _unpack_output(outb: np.ndarray) -> np.ndarray:
    q = outb[:, :ROWS, :O].reshape(NCORES * ROWS, O)
    rsc = np.ascontiguousarray(outb[:, :ROWS, O : O + 4]).view(np.float32)
    rsc = rsc.reshape(NCORES * ROWS, 1)
    probe = outb[:, ROWS, 0:8]

    # probe[1] == 131 iff float->u8 conversion truncates (floor for positives):
    # then u8 = floor(x*127/rmax + 128.5) = rtn(x*127/rmax) + 128 -> offset 128.
    # A round-to-nearest convert gives u8 = floor(x*127/rmax) + 129 for
    # non-tie values -> offset 128.5.
    off = 128.0 if int(probe[0, 1]) == 131 else 128.5
    scale = rsc.astype(np.float32) / 127.0
    out = (q.astype(np.float32) - off) * scale
    return out.reshape(B, N, O).astype(np.float32)
